# revision 3
# baseline (speedup 1.0000x reference)
"""CRF NLL (forward-algorithm partition function) on 8 Trainium2 NeuronCores.

Math: the reference computes  mean_b( logZ[b] - score[b] )  where
  logZ = logsumexp forward recursion over S=2048 steps with transition
  matrix T [L,L], emissions [B,S,L], and score is a pure gather path.

Device strategy (pure data parallel, batch sharded 8 ways, 16 seq/core):
  Linear-space recursion in layout [l (partitions), b (free)]:
      p_{t+1} = (expT^T @ p_t) * exp(em[:,t,:] - D)^T
  - PE matmul with stationary expT (lhsT, natural layout), rhs = p.
  - One DVE tensor_mul per step fuses the emission factor (PSUM -> SBUF).
  - D ~ mean log-growth per step keeps p O(1); every K_RESCALE steps an
    off-critical-path colsum (PE ones-matmul) + reciprocal (DVE) +
    broadcast (PE ones-matmul) rescales p exactly, with log(s) accumulated
    into c[b]; the scale is folded into a future emission tile so the
    critical chain stays exactly matmul -> tensor_mul per step.
  - Final: s_fin = expEnd @ p_S (PE), logZ_dev = ln(s_fin) + c (ACT+DVE).
Host: exp/transpose of emissions (pre), score gathers + mean (post).
"""

import os
from contextlib import ExitStack

import numpy as np

B, S, L = 128, 2048, 128
NCORES = 8
BS = B // NCORES  # 16 sequences per core
IGNORE = -100

D_SHIFT = 5.829        # expected per-step log growth (measured offline)
K_RESCALE = 64         # exact rescale cadence (steps)
APPLY_DELTA = 4        # steps between measuring colsum and applying 1/s
T_CHUNK = 256          # emission steps per DMA chunk

# test.py introspection
LAST_EXEC_TIME_NS = None
LAST_TRACE_PATH = None

_BUILT = {}


def _build(nsteps):
    """Build the Bass/Tile program for `nsteps` recursion steps (S-1 real)."""
    import concourse.bacc as bacc
    import concourse.tile as tile
    from concourse import mybir

    f32 = mybir.dt.float32
    Ln = mybir.ActivationFunctionType.Ln

    nc = bacc.Bacc(debug=False, name="crf_fwd")
    with tile.TileContext(nc) as tc:
        with ExitStack() as ctx:
            d_expT = nc.dram_tensor("expT", [L, L], f32, kind="ExternalInput")
            d_expEnd = nc.dram_tensor("expEnd", [L, 1], f32, kind="ExternalInput")
            d_p0 = nc.dram_tensor("p0", [L, BS], f32, kind="ExternalInput")
            d_E = nc.dram_tensor("emis", [L, nsteps, BS], f32, kind="ExternalInput")
            d_out = nc.dram_tensor("out", [1, BS], f32, kind="ExternalOutput")

            const = ctx.enter_context(tc.tile_pool(name="const", bufs=1))
            empool = ctx.enter_context(tc.tile_pool(name="empool", bufs=3))
            ppool = ctx.enter_context(tc.tile_pool(name="ppool", bufs=4))
            sclp = ctx.enter_context(tc.tile_pool(name="sclp", bufs=2))
            smalls = ctx.enter_context(tc.tile_pool(name="smalls", bufs=4))
            zpsum = ctx.enter_context(tc.tile_pool(name="zpsum", bufs=3, space="PSUM"))
            spsum = ctx.enter_context(tc.tile_pool(name="spsum", bufs=2, space="PSUM"))
            rpsum = ctx.enter_context(tc.tile_pool(name="rpsum", bufs=2, space="PSUM"))

            expT_sb = const.tile([L, L], f32)
            nc.sync.dma_start(out=expT_sb, in_=d_expT[:])
            expEnd_sb = const.tile([L, 1], f32)
            nc.sync.dma_start(out=expEnd_sb, in_=d_expEnd[:])
            ones_col = const.tile([L, 1], f32)
            nc.vector.memset(ones_col, 1.0)
            ones_row = const.tile([1, L], f32)
            nc.vector.memset(ones_row, 1.0)
            c_sb = const.tile([1, BS], f32)
            nc.vector.memset(c_sb, 0.0)

            p_cur = ppool.tile([L, BS], f32, tag="p")
            nc.sync.dma_start(out=p_cur, in_=d_p0[:])

            # apply-step -> PSUM broadcast tile of 1/s
            pending = {}
            em_tile = None
            chunk_lo = -1

            for t in range(1, nsteps + 1):
                i = t - 1  # emission index in d_E
                if i // T_CHUNK != chunk_lo:
                    chunk_lo = i // T_CHUNK
                    lo = chunk_lo * T_CHUNK
                    hi = min(lo + T_CHUNK, nsteps)
                    em_tile = empool.tile([L, T_CHUNK, BS], f32, tag="em")
                    nc.sync.dma_start(
                        out=em_tile[:, : hi - lo, :], in_=d_E[:, lo:hi, :]
                    )
                em_sl = em_tile[:, i % T_CHUNK, :]

                if t in pending:
                    rbc = pending.pop(t)
                    em_scaled = sclp.tile([L, BS], f32, tag="scl")
                    nc.vector.tensor_mul(em_scaled, em_sl, rbc)
                    em_sl = em_scaled

                z = zpsum.tile([L, BS], f32, tag="z")
                nc.tensor.matmul(z, lhsT=expT_sb, rhs=p_cur, start=True, stop=True)
                p_new = ppool.tile([L, BS], f32, tag="p")
                nc.vector.tensor_mul(p_new, z, em_sl)
                p_cur = p_new

                if t % K_RESCALE == 0 and t + APPLY_DELTA <= nsteps:
                    s_ps = spsum.tile([1, BS], f32, tag="s")
                    nc.tensor.matmul(
                        s_ps, lhsT=ones_col, rhs=p_cur, start=True, stop=True
                    )
                    r_sb = smalls.tile([1, BS], f32, tag="r")
                    nc.vector.reciprocal(r_sb, s_ps)
                    rbc = rpsum.tile([L, BS], f32, tag="rbc")
                    nc.tensor.matmul(
                        rbc, lhsT=ones_row, rhs=r_sb, start=True, stop=True
                    )
                    ln_s = smalls.tile([1, BS], f32, tag="lns")
                    nc.scalar.activation(ln_s, s_ps, Ln)
                    nc.vector.tensor_add(c_sb, c_sb, ln_s)
                    pending[t + APPLY_DELTA] = rbc

            s_fin = spsum.tile([1, BS], f32, tag="s")
            nc.tensor.matmul(s_fin, lhsT=expEnd_sb, rhs=p_cur, start=True, stop=True)
            ln_fin = smalls.tile([1, BS], f32, tag="lns")
            nc.scalar.activation(ln_fin, s_fin, Ln)
            outv = smalls.tile([1, BS], f32, tag="outv")
            nc.vector.tensor_add(outv, ln_fin, c_sb)
            nc.sync.dma_start(out=d_out[:], in_=outv)

    nc.compile()
    return nc


def _get_program(nsteps):
    if nsteps not in _BUILT:
        _BUILT[nsteps] = _build(nsteps)
    return _BUILT[nsteps]


def _prepare_in_maps(emissions, transitions, start_transitions, end_transitions,
                     nsteps=S - 1):
    """Host preprocessing -> (in_maps for 8 cores, c0[B])."""
    expT = np.exp(transitions, dtype=np.float32)                  # [l, l']
    expEnd = np.exp(end_transitions, dtype=np.float32).reshape(L, 1)

    alpha0 = start_transitions[None, :] + emissions[:, 0, :]      # [B, L] f32
    c0 = alpha0.max(axis=1)                                        # [B]
    p0_all = np.exp(alpha0 - c0[:, None]).T.astype(np.float32)     # [l, B]

    in_maps = []
    for k in range(NCORES):
        bs = slice(k * BS, (k + 1) * BS)
        # [l, t, b] = exp(em - D) transposed; steps 1..nsteps
        Ek = np.exp(
            np.ascontiguousarray(
                np.transpose(emissions[bs, 1 : nsteps + 1, :], (2, 1, 0))
            )
            - np.float32(D_SHIFT),
            dtype=np.float32,
        )
        in_maps.append(
            {
                "expT": expT,
                "expEnd": expEnd,
                "p0": np.ascontiguousarray(p0_all[:, bs]),
                "emis": Ek,
            }
        )
    return in_maps, c0


def _forward_device(emissions, transitions, start_transitions, end_transitions,
                    nsteps=S - 1):
    """Run the device recursion; returns logZ [B] float64."""
    from concourse.bass_utils import run_bass_kernel_spmd

    global LAST_EXEC_TIME_NS, LAST_TRACE_PATH

    in_maps, c0 = _prepare_in_maps(
        emissions, transitions, start_transitions, end_transitions, nsteps
    )
    nc = _get_program(nsteps)
    trace = os.environ.get("CRF_TRACE", "") == "1"
    res = run_bass_kernel_spmd(
        nc, in_maps, core_ids=list(range(NCORES)), trace=trace
    )
    LAST_EXEC_TIME_NS = res.exec_time_ns
    if res.instructions_and_trace is not None:
        LAST_TRACE_PATH = res.instructions_and_trace[1]

    out = np.concatenate([res.results[k]["out"][0] for k in range(NCORES)])
    return out.astype(np.float64) + c0.astype(np.float64) + D_SHIFT * nsteps


def _score_host(emissions, mask, tags, transitions, start_transitions,
                end_transitions):
    """Gold path score, matching reference._crf_nll's gather path. float64."""
    em = emissions.astype(np.float64)
    T = transitions.astype(np.float64)
    startT = start_transitions.astype(np.float64)
    endT = end_transitions.astype(np.float64)

    valid = tags != IGNORE
    tags_safe = np.where(valid, tags, 0).astype(np.int64)
    vf = valid.astype(np.float64)

    score = startT[tags_safe[:, 0]] * vf[:, 0]
    prev_t = tags_safe[:, :-1]
    curr_t = tags_safe[:, 1:]
    trans_sc = T[prev_t, curr_t]
    em_sc = np.take_along_axis(em[:, 1:, :], curr_t[:, :, None], axis=2)[..., 0]
    score = score + np.sum((trans_sc + em_sc) * vf[:, 1:], axis=1)

    pos = np.arange(tags.shape[1])
    last_idx = np.max(np.where(valid, pos[None, :], -1), axis=1)
    last_tag = tags_safe[np.arange(tags.shape[0]), np.clip(last_idx, 0, S - 1)]
    score = score + np.where(last_idx >= 0, endT[last_tag], 0.0)
    return score


def _forward_numpy(emissions, mask, transitions, start_transitions,
                   end_transitions):
    """Fallback exact forward recursion (used only if mask isn't all ones)."""
    em = emissions.astype(np.float64)
    T = transitions.astype(np.float64)
    alpha = start_transitions.astype(np.float64)[None, :] + em[:, 0, :]
    for t in range(1, em.shape[1]):
        m = alpha.max(axis=1, keepdims=True)
        new = m + np.log(np.exp(alpha - m) @ np.exp(T)) + em[:, t, :]
        alpha = np.where(mask[:, t][:, None], new, alpha)
    m = alpha.max(axis=1, keepdims=True)
    return (
        m[:, 0]
        + np.log(
            np.exp(alpha - m) @ np.exp(end_transitions.astype(np.float64))
        )
    )


def kernel(emissions, mask, tags, transitions, start_transitions,
           end_transitions):
    emissions = np.asarray(emissions, dtype=np.float32)
    mask = np.asarray(mask)
    tags = np.asarray(tags)
    transitions = np.asarray(transitions, dtype=np.float32)
    start_transitions = np.asarray(start_transitions, dtype=np.float32)
    end_transitions = np.asarray(end_transitions, dtype=np.float32)

    if bool(mask.all()):
        logz = _forward_device(
            emissions, transitions, start_transitions, end_transitions
        )
    else:
        logz = _forward_numpy(
            emissions, mask, transitions, start_transitions, end_transitions
        )

    score = _score_host(
        emissions, mask, tags, transitions, start_transitions, end_transitions
    )
    return np.asarray(np.mean(logz - score), dtype=np.float32)
